# revision 32
# baseline (speedup 1.0000x reference)
"""GAT single-head forward on 8 Trainium2 NeuronCores (Bass/Tile).

Math (per reference):
    h   = X @ W + b                      [N, 128]
    f1  = h @ v0, f2 = h @ v1            [N]
    logits = adj * (f1[:,None] + f2[None,:])   (adj entries are exactly 0/1)
    vals = sigmoid(logits) - 0.5
    masked softmax over row edges; out = probs @ h

Key identities used on device:
  * On edges (adj==1) the softmax weight is phi(s) = exp(sigmoid(s)) with
    s = f1_i + f2_j (constant shifts cancel; exp never overflows).
  * phi(s) ~= A + B*tanh(LAM*s + MU) to 5.7e-4 max relative error over the
    full attainable s range. The softmax ratio cancels the global factor B,
    so on device only  et = (tanh(LAM*s+MU) + A/B) * adj  is needed:
        probs = et / rowsum(et).
    This replaces the baseline's tanh+exp double activation pass (the 143us
    kernel's bottleneck: ACT busy 87%) with a SINGLE tanh pass.
  * The tanh argument is built for free by the ACT unit itself: the input is
    f1 (LAM-prescaled, MU-shifted) broadcast across partitions, and LAM*f2
    rides the per-partition activation bias, one [P,1] column per j-chunk.
    No separate dense pre-add pass exists at all.
  * The +C0 shift is one grouped in-place tensor_scalar per group (~0.29
    ns/col); the mask is a per-chunk 1024-wide tensor_tensor multiply
    (~0.67 ns/col). Measured on HW: a fused scalar_tensor_tensor runs ~1.6x
    slower than TT, 8192-wide TT runs ~1.6x slower than 1024-wide, and an
    fp8 second operand slows DVE in-situ - hence fp16 adj, narrow TTs, and
    the TS+TT split.
  * A ones-column appended to h turns the softmax denominator into one extra
    matmul output column.

Sharding: rows of adj across the 8 cores (1024 rows each). node_feats is
small and replicated; every core computes the full projected h locally -
no collectives.

Per-core layout: adj block transposed ([j=source node on partitions, i=own
rows on free dim]) so the aggregate probs@h contracts over the partition
dim. adj streams as fp16 group tiles (triple-buffered) whose DMAs are
emitted inside each group front; the first fronts are emitted before the
bulk feature loads so their adj tiles don't queue behind 4MB of xt1.

PSUM: the 8 output accumulators pack two per bank (129 cols at offsets
0/256) in 4 banks, coexisting with the 2x2-bank projection pool. A matmul
with start=True wipes the WHOLE destination bank (verified on HW), so the
banks are wiped once by dummy 1-col matmuls and all aggregation matmuls run
start=False, landing via the cleared has_written bits. Consecutive
aggregation matmuls walk banks 0123 0123 (same-bank accumulation would
serialize the PE).

Schedule shape (engines are in-order; emission order seeds the queues):
  preamble -> [weights/features DMA | f1 path | f2 head] -> group fronts
  (adj DMA + 8 bias-trick tanhs each) pipelined 3 deep against group backs
  (grouped +C0, per-chunk mask TTs, 8 aggregation matmuls per chunk), with
  the h-projection mini-batches interleaved ahead -> epilogue divide and a
  4-way-split output store.
"""

import os

import numpy as np
import ml_dtypes

import concourse.mybir as mybir
import concourse.tile as tile
from concourse import bacc
from concourse.bass_utils import run_bass_kernel_spmd

F32 = mybir.dt.float32
F16 = mybir.dt.float16
BF16 = mybir.dt.bfloat16
F8 = mybir.dt.float8e4
AF = mybir.ActivationFunctionType
ALU = mybir.AluOpType

N, C_IN, C_OUT = 8192, 256, 128
NCORES = 8
ROWS = N // NCORES          # 1024 rows of adj per core
P = 128
NT = N // P                 # 64 node tiles (also the j-chunks)
NI = ROWS // P              # 8 output row-tiles per core
KC = [128, 128, 1]          # contraction chunks of K=257 (X.T rows + ones row)
WCOLS = C_OUT + 3           # [W | ones-hack | LAM*w0 | LAM*w1]
HCOLS = C_OUT + 1           # h plus the ones column
TINY = float(np.finfo(np.float32).tiny)
BANK = 512                  # PSUM bank, fp32 elements

# phi(s) = exp(sigmoid(s)) ~= A + B*tanh(LAM*s + MU); only C0 = A/B survives
# the softmax normalization.
LAM = 0.5082714
MU = -0.24995726
C0 = 1.85905591 / 0.85894722

# activation groups: j-chunks per pipeline stage. Small leading groups start
# the ACT chain early; small trailing groups shorten the tail.
GROUPS = [2, 4] + [8] * 6 + [4, 4, 2]
# groups whose chunks compute tanh(f1rep + f2) directly via the ACT
# per-partition bias (no DVE preadd, ~+155ns/chunk on ACT): balances the
# in-order DVE queue (preadds + grouped add/mask) against ACT.
BIAS_GROUPS = set(range(16))

_CACHE: dict = {}


def _build_nc(b_zero=True):
    nc = bacc.Bacc(
        "TRN2", target_bir_lowering=False, debug=False, num_devices=NCORES
    )
    xt1 = nc.dram_tensor("xt1", [257, N], F16, kind="ExternalInput").ap()
    xt1l = nc.dram_tensor("xt1l", [257, ROWS], F16, kind="ExternalInput").ap()
    wext = nc.dram_tensor("wext", [257, WCOLS], F16, kind="ExternalInput").ap()
    adjt = nc.dram_tensor("adjt", [N, ROWS], F16, kind="ExternalInput").ap()
    out = nc.dram_tensor("out", [ROWS, C_OUT], F32, kind="ExternalOutput").ap()

    with tile.TileContext(nc) as tc:
        _emit(tc, nc, xt1, xt1l, wext, adjt, out, b_zero)
    nc.compile()
    return nc


def _emit(tc, nc, xt1, xt1l, wext, adjt, out, b_zero):
    from contextlib import ExitStack

    # with b == 0 the K=1 "ones row" contraction chunk only contributes the
    # constant-one column of h_ext (done with a strided memset instead) and
    # zero constants to f1/f2 -- skip it entirely.
    nkc = 2 if b_zero else 3

    with ExitStack() as ctx:
        # ---- persistent tiles ----
        persist = ctx.enter_context(tc.tile_pool(name="persist", bufs=1))
        h16_all = persist.tile([P, NT * HCOLS], F16, tag="h16")   # [128, 8256]
        f2h_all = persist.tile([P, NT], F32, tag="f2h")           # LAM*f2 per j
        f1rep = persist.tile([P, ROWS], F16, tag="f1rep")         # LAM*f1+MU
        zero1 = persist.tile([P, 1], F32, tag="zero1")
        nc.vector.memset(zero1[:], 0.0)
        mu1 = persist.tile([P, 1], F32, tag="mu1")
        nc.vector.memset(mu1[:], MU)
        if b_zero:
            # constant-one column of every h_ext tile (replaces the K=1
            # bias matmul chunk)
            nc.vector.memset(
                h16_all[:].rearrange("p (t c) -> p t c", c=HCOLS)[
                    :, :, C_OUT : C_OUT + 1
                ],
                1.0,
            )

        xtp = ctx.enter_context(tc.tile_pool(name="xt", bufs=1))

        # ---- input loads ----
        # small inputs first so the f1 path clears quickly; adj chunk DMAs
        # round-robin across queues, interleaved with the xt1 column slices
        # so early chunks land before their mask-STT needs them.
        offs = [0, 128, 256]
        xts = [
            xtp.tile([KC[k], N], F16, name=f"xtsb{k}", tag=f"xt{k}")
            for k in range(nkc)
        ]
        SUBS = [0, 1024, 3072, 5120, N]
        wes, xls = [], []
        off = 0
        for k in range(nkc):
            kc = KC[k]
            wx_sb = xtp.tile([kc, WCOLS + ROWS], F16, name=f"wx{k}", tag=f"wx{k}")
            nc.sync.dma_start(wx_sb[:, 0:WCOLS], wext[off : off + kc, :])
            for pc in range(4):
                nc.sync.dma_start(
                    wx_sb[:, WCOLS + pc * 256 : WCOLS + (pc + 1) * 256],
                    xt1l[off : off + kc, pc * 256 : (pc + 1) * 256],
                )
            wes.append(wx_sb[:, 0:WCOLS])
            xls.append(wx_sb[:, WCOLS:])
            off += kc
        for k in range(nkc):
            if KC[k] == P:
                nc.sync.dma_start(
                    xts[k][:, 0 : SUBS[1]],
                    xt1[offs[k] : offs[k] + KC[k], 0 : SUBS[1]],
                )

        adjs = adjt.rearrange("(q p) i -> p q i", p=P)


        # ---- f1 path: LAM*f1 + MU for this core's rows, replicated across
        # all partitions by a matmul whose stationary operand is the LAM*w0
        # column broadcast across the 128 PE columns ----
        with tc.tile_pool(name="pf", bufs=1, space="PSUM") as pfp:
            prep = pfp.tile([P, ROWS], F32, tag="prep")
            for k in range(nkc):
                for nh in range(ROWS // 512):
                    nc.tensor.matmul(
                        prep[:, nh * 512 : (nh + 1) * 512],
                        wes[k][:, C_OUT + 1 : C_OUT + 2].to_broadcast(
                            (KC[k], P)
                        ),
                        xls[k][:, nh * 512 : (nh + 1) * 512],
                        start=(k == 0),
                        stop=(k == nkc - 1),
                    )
            nc.scalar.activation(
                f1rep[:], prep[:], AF.Identity, bias=mu1[:], scale=1.0
            )

        # ---- f2 head start: LAM*f2 for the first 8 j-chunks via tiny
        # direct matmuls so activation groups 0/1 don't wait for the
        # h-projection pipeline ----
        F2HEAD = 8
        with tc.tile_pool(name="pf2", bufs=1, space="PSUM") as pf2p:
            pt = pf2p.tile([P, NI * BANK], F32, tag="pt")
            pt3 = pt[:].rearrange("p (t w) -> p t w", w=BANK)
            for q in range(F2HEAD):
                w = (q % NI) * BANK
                for k in range(nkc):
                    nc.tensor.matmul(
                        pt[:, w : w + 1],
                        xts[k][:, q * P : (q + 1) * P],
                        wes[k][:, C_OUT + 2 : C_OUT + 3],
                        start=(k == 0),
                        stop=(k == nkc - 1),
                    )
                if q == 1:
                    # group 0's two columns drain immediately so its
                    # preadds (and the whole activation chain) start early
                    nc.vector.tensor_copy(
                        f2h_all[:, 0:2], pt3[:, 0:2, 0:1]
                    )
            nc.vector.tensor_copy(
                f2h_all[:, 2:F2HEAD], pt3[:, 2:F2HEAD, 0:1]
            )

        # ---- main-loop pools ----
        sup = ctx.enter_context(tc.tile_pool(name="sup", bufs=1))
        adjp = ctx.enter_context(tc.tile_pool(name="adjp", bufs=1))
        etp = ctx.enter_context(tc.tile_pool(name="etp", bufs=1))
        obp = ctx.enter_context(tc.tile_pool(name="ob", bufs=2))

        # aggregate accumulators: 8 row-tiles x 129 cols packed two per
        # PSUM bank (129 <= 256) -> 4 banks, sharing ONE pool with the
        # projection tiles (2 x 2 banks) so aggregation interleaves with
        # projection without bank collisions.
        pop = ctx.enter_context(tc.tile_pool(name="po", bufs=1, space="PSUM"))
        po_all = pop.tile([P, 4 * BANK], F32, tag="poall")
        pouts = [po_all[:, i * 256 : i * 256 + HCOLS] for i in range(NI)]
        # start=True wipes the WHOLE destination bank (data + has_written),
        # so packed accumulators can't each carry their own start. Wipe each
        # bank once with a dummy 1-col matmul; the aggregation then runs
        # start=False throughout (first write per element lands via the
        # cleared has_written bits).
        z16 = persist.tile([P, 1], F16, tag="z16")
        nc.vector.memset(z16[:], 0.0)
        for bk in range(4):
            nc.tensor.matmul(
                po_all[:, bk * BANK : bk * BANK + 1],
                f1rep[:, 0:P],
                z16[:],
                start=True,
                stop=True,
            )

        group_q0 = []
        q0 = 0
        for gsz in GROUPS:
            group_q0.append(q0)
            q0 += gsz

        deferred = []  # groups produced but not yet masked/aggregated

        def emit_group_front(g):
            """Produce w = tanh(LAM*s + MU) for the group: either DVE
            preadds + one fused tanh, or per-chunk ACT-bias tanh (f2 rides
            the per-partition bias, no preadd)."""
            gsz = GROUPS[g]
            q0 = group_q0[g]
            s_sup = sup.tile([P, gsz * ROWS], F16, tag="s", bufs=3, name=f"s{g}")
            at = adjp.tile(
                [P, gsz * ROWS], F16, tag="at", bufs=3, name=f"at{g}"
            )
            at3 = at[:].rearrange("p (q i) -> p q i", i=ROWS)
            for qq in range(gsz):
                nc.sync.dma_start(
                    at3[:, qq : qq + 1, :], adjs[:, q0 + qq : q0 + qq + 1, :]
                )
            if g in BIAS_GROUPS:
                for qq in range(gsz):
                    q = q0 + qq
                    nc.scalar.activation(
                        s_sup[:, qq * ROWS : (qq + 1) * ROWS],
                        f1rep[:],
                        AF.Tanh,
                        bias=f2h_all[:, q : q + 1],
                    )
            else:
                for qq in range(gsz):
                    q = q0 + qq
                    nc.vector.tensor_scalar_add(
                        s_sup[:, qq * ROWS : (qq + 1) * ROWS],
                        f1rep[:],
                        f2h_all[:, q : q + 1],
                    )
                nc.scalar.activation(s_sup[:], s_sup[:], AF.Tanh, bias=zero1[:])
            return {"g": g, "gsz": gsz, "q0": q0, "w": s_sup, "at": at}

        def emit_group_back(fr):
            """grouped +C0 then mask multiply (both in place on the group
            tile; the fp8 adj auto-converts), then aggregate matmuls."""
            gsz, q0, w_sup = fr["gsz"], fr["q0"], fr["w"]
            et = etp.tile(
                [P, gsz * ROWS], F16, tag="et", bufs=2, name=f"et{q0}"
            )
            nc.vector.tensor_scalar_add(w_sup[:], w_sup[:], float(C0))
            for qq in range(gsz):
                sl = slice(qq * ROWS, (qq + 1) * ROWS)
                nc.vector.tensor_mul(et[:, sl], w_sup[:, sl], fr["at"][:, sl])
            for qq in range(gsz):
                q = q0 + qq
                rhs = h16_all[:, q * HCOLS : (q + 1) * HCOLS]
                # consecutive matmuls must hit different PSUM banks (same-
                # bank accumulation serializes the PE): walk banks 0123 0123
                for it in (0, 2, 4, 6, 1, 3, 5, 7):
                    nc.tensor.matmul(
                        pouts[it],
                        et[:, qq * ROWS + it * P : qq * ROWS + (it + 1) * P],
                        rhs,
                        start=False,
                        stop=(q == NT - 1),
                    )

        # ---- h-projection in mini-batches of 2 node tiles (double-buffered
        # 2-bank PSUM tiles), with group fronts/backs interleaved so the
        # tanh chain, mask ops and aggregation all overlap projection ----
        next_group = 0
        hc = C_OUT if b_zero else HCOLS
        # groups covered by the f2 head start don't wait for projection
        while (
            next_group < len(GROUPS)
            and group_q0[next_group] + GROUPS[next_group] <= F2HEAD
        ):
            deferred.append(emit_group_front(next_group))
            next_group += 1
        # bulk xt1 slices AFTER the head-start fronts so the first adj
        # group tiles don't queue behind 4MB of features
        for c in range(1, len(SUBS) - 1):
            for k in range(nkc):
                if KC[k] != P:
                    if c == 1:
                        nc.sync.dma_start(
                            xts[k][:], xt1[offs[k] : offs[k] + KC[k], :]
                        )
                    continue
                nc.sync.dma_start(
                    xts[k][:, SUBS[c] : SUBS[c + 1]],
                    xt1[offs[k] : offs[k] + KC[k], SUBS[c] : SUBS[c + 1]],
                )
        if True:
            for mb in range(NT // 2):  # mini-batches of 2 node tiles
                ph = pop.tile(
                    [P, 2 * BANK], F32, tag="ph", bufs=2, name=f"ph{mb}"
                )
                nt0 = 2 * mb
                for k in range(nkc):
                    nc.tensor.matmul(
                        ph[:, 0:WCOLS],
                        xts[k][:, nt0 * P : (nt0 + 1) * P],
                        wes[k][:],
                        start=(k == 0),
                        stop=(k == nkc - 1),
                    )
                    nc.tensor.matmul(
                        ph[:, BANK : BANK + WCOLS],
                        xts[k][:, (nt0 + 1) * P : (nt0 + 2) * P],
                        wes[k][:],
                        start=(k == 0),
                        stop=(k == nkc - 1),
                    )
                # drain h (+f2 col) of the 2 fresh tiles
                src = ph[:].rearrange("p (b w) -> p b w", b=2)
                dst_h = h16_all[:, nt0 * HCOLS : (nt0 + 2) * HCOLS].rearrange(
                    "p (b w) -> p b w", b=2
                )
                nc.vector.tensor_copy(dst_h[:, :, 0:hc], src[:, :, 0:hc])
                if nt0 >= 8:  # first 8 f2 columns came from the head start
                    nc.vector.tensor_copy(
                        f2h_all[:, nt0 : nt0 + 2],
                        src[:, :, C_OUT + 2 : C_OUT + 3],
                    )
                done = 2 * (mb + 1)  # chunks fully drained
                while (
                    next_group < len(GROUPS)
                    and group_q0[next_group] + GROUPS[next_group] <= done
                    and len(deferred) < 3
                ):
                    deferred.append(emit_group_front(next_group))
                    next_group += 1

        # ---- flush remaining groups ----
        while deferred or next_group < len(GROUPS):
            if next_group < len(GROUPS) and len(deferred) < 3:
                deferred.append(emit_group_front(next_group))
                next_group += 1
            if deferred:
                emit_group_back(deferred.pop(0))

        # ---- epilogue: divide by clamped denominator, one batched store ----
        ob_all = obp.tile([P, NI * C_OUT], F32, tag="oball")
        po3 = po_all[:].rearrange("p (t w) -> p t w", w=256)
        dm = obp.tile([P, NI], F32, tag="dm")
        nc.vector.tensor_scalar_max(
            dm[:], po3[:, :, C_OUT : C_OUT + 1], TINY
        )
        rc = obp.tile([P, NI], F32, tag="rc")
        nc.vector.reciprocal(rc[:], dm[:])
        for it in range(NI):
            # alternate engines: ACT is idle after the last tanh
            if it % 2 == 0:
                nc.vector.tensor_scalar_mul(
                    ob_all[:, it * C_OUT : (it + 1) * C_OUT],
                    po_all[:, it * 256 : it * 256 + C_OUT],
                    rc[:, it : it + 1],
                )
            else:
                nc.scalar.mul(
                    ob_all[:, it * C_OUT : (it + 1) * C_OUT],
                    po_all[:, it * 256 : it * 256 + C_OUT],
                    rc[:, it : it + 1],
                )
        outr = out.rearrange("(t p) c -> p t c", p=P)
        obr = ob_all[:].rearrange("p (t c) -> p t c", c=C_OUT)
        for tp in range(4):
            nc.sync.dma_start(
                outr[:, 2 * tp : 2 * (tp + 1), :],
                obr[:, 2 * tp : 2 * (tp + 1), :],
            )


def _prep_inputs(node_feats, adj_matrix, W, b, v0, v1):
    X = np.ascontiguousarray(node_feats, dtype=np.float32)
    W = np.asarray(W, dtype=np.float32)
    b = np.asarray(b, dtype=np.float32)
    v0 = np.asarray(v0, dtype=np.float32)
    v1 = np.asarray(v1, dtype=np.float32)

    w0l = (LAM * (W.astype(np.float64) @ v0.astype(np.float64))).astype(np.float32)
    w1l = (LAM * (W.astype(np.float64) @ v1.astype(np.float64))).astype(np.float32)
    c0l = np.float32(LAM * float(b.astype(np.float64) @ v0.astype(np.float64)))
    c1l = np.float32(LAM * float(b.astype(np.float64) @ v1.astype(np.float64)))

    XT1 = np.empty((257, N), np.float32)
    XT1[:256] = X.T
    XT1[256] = 1.0

    WE = np.zeros((257, WCOLS), np.float32)
    WE[:256, :C_OUT] = W
    WE[256, :C_OUT] = b
    WE[256, C_OUT] = 1.0          # makes h_ext column 128 identically 1
    WE[:256, C_OUT + 1] = w0l
    WE[256, C_OUT + 1] = c0l
    WE[:256, C_OUT + 2] = w1l
    WE[256, C_OUT + 2] = c1l

    XT1h = XT1.astype(np.float16)
    WEh = WE.astype(np.float16)
    A16 = np.asarray(adj_matrix, dtype=np.float16)

    in_maps = []
    for c in range(NCORES):
        in_maps.append(
            {
                "xt1": XT1h,
                "xt1l": np.ascontiguousarray(XT1h[:, c * ROWS : (c + 1) * ROWS]),
                "wext": WEh,
                "adjt": np.ascontiguousarray(
                    A16[c * ROWS : (c + 1) * ROWS, :].T
                ),
            }
        )
    return in_maps


def _run(in_maps, trace=False, b_zero=True):
    key = f"nc_b{int(b_zero)}"
    if key not in _CACHE:
        _CACHE[key] = _build_nc(b_zero=b_zero)
    nc = _CACHE[key]
    res = run_bass_kernel_spmd(
        nc, in_maps, core_ids=list(range(NCORES)), trace=trace
    )
    full = np.concatenate(
        [res.results[c]["out"] for c in range(NCORES)], axis=0
    ).astype(np.float32)
    return full, res


def kernel(node_feats, adj_matrix, W, b, v0, v1):
    in_maps = _prep_inputs(node_feats, adj_matrix, W, b, v0, v1)
    trace = bool(int(os.environ.get("GAT_TRACE", "0")))
    b_zero = not bool(np.any(np.asarray(b)))
    full, _ = _run(in_maps, trace=trace, b_zero=b_zero)
    return full


# revision 33
# speedup vs baseline: 1.0050x; 1.0050x over previous
"""GAT single-head forward on 8 Trainium2 NeuronCores (Bass/Tile).

Math (per reference):
    h   = X @ W + b                      [N, 128]
    f1  = h @ v0, f2 = h @ v1            [N]
    logits = adj * (f1[:,None] + f2[None,:])   (adj entries are exactly 0/1)
    vals = sigmoid(logits) - 0.5
    masked softmax over row edges; out = probs @ h

Key identities used on device:
  * On edges (adj==1) the softmax weight is phi(s) = exp(sigmoid(s)) with
    s = f1_i + f2_j (constant shifts cancel; exp never overflows).
  * phi(s) ~= A + B*tanh(LAM*s + MU) to 5.7e-4 max relative error over the
    full attainable s range. The softmax ratio cancels the global factor B,
    so on device only  et = (tanh(LAM*s+MU) + A/B) * adj  is needed:
        probs = et / rowsum(et).
    This replaces the baseline's tanh+exp double activation pass (the 143us
    kernel's bottleneck: ACT busy 87%) with a SINGLE tanh pass.
  * The tanh argument is built for free by the ACT unit itself: the input is
    f1 (LAM-prescaled, MU-shifted) broadcast across partitions, and LAM*f2
    rides the per-partition activation bias, one [P,1] column per j-chunk.
    No separate dense pre-add pass exists at all.
  * The +C0 shift is one grouped in-place tensor_scalar per group (~0.29
    ns/col); the mask is a per-chunk 1024-wide tensor_tensor multiply
    (~0.67 ns/col). Measured on HW: a fused scalar_tensor_tensor runs ~1.6x
    slower than TT, 8192-wide TT runs ~1.6x slower than 1024-wide, and an
    fp8 second operand slows DVE in-situ - hence fp16 adj, narrow TTs, and
    the TS+TT split.
  * A ones-column appended to h turns the softmax denominator into one extra
    matmul output column.

Sharding: rows of adj across the 8 cores (1024 rows each). node_feats is
small and replicated; every core computes the full projected h locally -
no collectives.

Per-core layout: adj block transposed ([j=source node on partitions, i=own
rows on free dim]) so the aggregate probs@h contracts over the partition
dim. adj streams as fp16 group tiles (triple-buffered) whose DMAs are
emitted inside each group front; the first fronts are emitted before the
bulk feature loads so their adj tiles don't queue behind 4MB of xt1.

PSUM: the 8 output accumulators pack two per bank (129 cols at offsets
0/256) in 4 banks, coexisting with the 2x2-bank projection pool. A matmul
with start=True wipes the WHOLE destination bank (verified on HW), so the
banks are wiped once by dummy 1-col matmuls and all aggregation matmuls run
start=False, landing via the cleared has_written bits. Consecutive
aggregation matmuls walk banks 0123 0123 (same-bank accumulation would
serialize the PE).

Schedule shape (engines are in-order; emission order seeds the queues):
  preamble -> [weights/features DMA | f1 path | f2 head] -> group fronts
  (adj DMA + 8 bias-trick tanhs each) pipelined 3 deep against group backs
  (grouped +C0, per-chunk mask TTs, 8 aggregation matmuls per chunk), with
  the h-projection mini-batches interleaved ahead -> epilogue divide and a
  4-way-split output store.
"""

import os

import numpy as np
import ml_dtypes

import concourse.mybir as mybir
import concourse.tile as tile
from concourse import bacc
from concourse.bass_utils import run_bass_kernel_spmd

F32 = mybir.dt.float32
F16 = mybir.dt.float16
BF16 = mybir.dt.bfloat16
F8 = mybir.dt.float8e4
AF = mybir.ActivationFunctionType
ALU = mybir.AluOpType

N, C_IN, C_OUT = 8192, 256, 128
NCORES = 8
ROWS = N // NCORES          # 1024 rows of adj per core
P = 128
NT = N // P                 # 64 node tiles (also the j-chunks)
NI = ROWS // P              # 8 output row-tiles per core
KC = [128, 128, 1]          # contraction chunks of K=257 (X.T rows + ones row)
WCOLS = C_OUT + 3           # [W | ones-hack | LAM*w0 | LAM*w1]
HCOLS = C_OUT + 1           # h plus the ones column
TINY = float(np.finfo(np.float32).tiny)
BANK = 512                  # PSUM bank, fp32 elements

# phi(s) = exp(sigmoid(s)) ~= A + B*tanh(LAM*s + MU); only C0 = A/B survives
# the softmax normalization.
LAM = 0.5082714
MU = -0.24995726
C0 = 1.85905591 / 0.85894722

# activation groups: j-chunks per pipeline stage. Small leading groups start
# the ACT chain early; small trailing groups shorten the tail.
GROUPS = [2, 4] + [8] * 6 + [4, 4, 2]
# groups whose chunks compute tanh(f1rep + f2) directly via the ACT
# per-partition bias (no DVE preadd, ~+155ns/chunk on ACT): balances the
# in-order DVE queue (preadds + grouped add/mask) against ACT.
BIAS_GROUPS = set(range(16))

_CACHE: dict = {}


def _build_nc(b_zero=True):
    nc = bacc.Bacc(
        "TRN2", target_bir_lowering=False, debug=False, num_devices=NCORES
    )
    xt1 = nc.dram_tensor("xt1", [257, N], F16, kind="ExternalInput").ap()
    xt1l = nc.dram_tensor("xt1l", [257, ROWS], F16, kind="ExternalInput").ap()
    wext = nc.dram_tensor("wext", [257, WCOLS], F16, kind="ExternalInput").ap()
    adjt = nc.dram_tensor("adjt", [N, ROWS], F16, kind="ExternalInput").ap()
    out = nc.dram_tensor("out", [ROWS, C_OUT], F32, kind="ExternalOutput").ap()

    with tile.TileContext(nc) as tc:
        _emit(tc, nc, xt1, xt1l, wext, adjt, out, b_zero)
    nc.compile()
    return nc


def _emit(tc, nc, xt1, xt1l, wext, adjt, out, b_zero):
    from contextlib import ExitStack

    # with b == 0 the K=1 "ones row" contraction chunk only contributes the
    # constant-one column of h_ext (done with a strided memset instead) and
    # zero constants to f1/f2 -- skip it entirely.
    nkc = 2 if b_zero else 3

    with ExitStack() as ctx:
        # ---- persistent tiles ----
        persist = ctx.enter_context(tc.tile_pool(name="persist", bufs=1))
        h16_all = persist.tile([P, NT * HCOLS], F16, tag="h16")   # [128, 8256]
        f2h_all = persist.tile([P, NT], F32, tag="f2h")           # LAM*f2 per j
        f1rep = persist.tile([P, ROWS], F16, tag="f1rep")         # LAM*f1+MU
        zero1 = persist.tile([P, 1], F32, tag="zero1")
        nc.vector.memset(zero1[:], 0.0)
        mu1 = persist.tile([P, 1], F32, tag="mu1")
        nc.vector.memset(mu1[:], MU)
        if b_zero:
            # constant-one column of every h_ext tile (replaces the K=1
            # bias matmul chunk)
            nc.vector.memset(
                h16_all[:].rearrange("p (t c) -> p t c", c=HCOLS)[
                    :, :, C_OUT : C_OUT + 1
                ],
                1.0,
            )

        xtp = ctx.enter_context(tc.tile_pool(name="xt", bufs=1))

        # ---- input loads ----
        # small inputs first so the f1 path clears quickly; adj chunk DMAs
        # round-robin across queues, interleaved with the xt1 column slices
        # so early chunks land before their mask-STT needs them.
        offs = [0, 128, 256]
        xts = [
            xtp.tile([KC[k], N], F16, name=f"xtsb{k}", tag=f"xt{k}")
            for k in range(nkc)
        ]
        SUBS = [0, 1024, 3072, 5120, N]
        wes, xls = [], []
        off = 0
        for k in range(nkc):
            kc = KC[k]
            wx_sb = xtp.tile([kc, WCOLS + ROWS], F16, name=f"wx{k}", tag=f"wx{k}")
            nc.sync.dma_start(wx_sb[:, 0:WCOLS], wext[off : off + kc, :])
            nc.sync.dma_start(wx_sb[:, WCOLS:], xt1l[off : off + kc, :])
            wes.append(wx_sb[:, 0:WCOLS])
            xls.append(wx_sb[:, WCOLS:])
            off += kc
        for k in range(nkc):
            if KC[k] == P:
                nc.sync.dma_start(
                    xts[k][:, 0 : SUBS[1]],
                    xt1[offs[k] : offs[k] + KC[k], 0 : SUBS[1]],
                )

        adjs = adjt.rearrange("(q p) i -> p q i", p=P)


        # ---- f1 path: LAM*f1 + MU for this core's rows, replicated across
        # all partitions by a matmul whose stationary operand is the LAM*w0
        # column broadcast across the 128 PE columns ----
        with tc.tile_pool(name="pf", bufs=1, space="PSUM") as pfp:
            prep = pfp.tile([P, ROWS], F32, tag="prep")
            for k in range(nkc):
                for nh in range(ROWS // 512):
                    nc.tensor.matmul(
                        prep[:, nh * 512 : (nh + 1) * 512],
                        wes[k][:, C_OUT + 1 : C_OUT + 2].to_broadcast(
                            (KC[k], P)
                        ),
                        xls[k][:, nh * 512 : (nh + 1) * 512],
                        start=(k == 0),
                        stop=(k == nkc - 1),
                    )
            nc.scalar.activation(
                f1rep[:], prep[:], AF.Identity, bias=mu1[:], scale=1.0
            )

        # ---- f2 head start: LAM*f2 for the first 8 j-chunks via tiny
        # direct matmuls so activation groups 0/1 don't wait for the
        # h-projection pipeline ----
        F2HEAD = 8
        with tc.tile_pool(name="pf2", bufs=1, space="PSUM") as pf2p:
            pt = pf2p.tile([P, NI * BANK], F32, tag="pt")
            pt3 = pt[:].rearrange("p (t w) -> p t w", w=BANK)
            for q in range(F2HEAD):
                w = (q % NI) * BANK
                for k in range(nkc):
                    nc.tensor.matmul(
                        pt[:, w : w + 1],
                        xts[k][:, q * P : (q + 1) * P],
                        wes[k][:, C_OUT + 2 : C_OUT + 3],
                        start=(k == 0),
                        stop=(k == nkc - 1),
                    )
                if q == 1:
                    # group 0's two columns drain immediately so its
                    # preadds (and the whole activation chain) start early
                    nc.vector.tensor_copy(
                        f2h_all[:, 0:2], pt3[:, 0:2, 0:1]
                    )
            nc.vector.tensor_copy(
                f2h_all[:, 2:F2HEAD], pt3[:, 2:F2HEAD, 0:1]
            )

        # ---- main-loop pools ----
        sup = ctx.enter_context(tc.tile_pool(name="sup", bufs=1))
        adjp = ctx.enter_context(tc.tile_pool(name="adjp", bufs=1))
        etp = ctx.enter_context(tc.tile_pool(name="etp", bufs=1))
        obp = ctx.enter_context(tc.tile_pool(name="ob", bufs=2))

        # aggregate accumulators: 8 row-tiles x 129 cols packed two per
        # PSUM bank (129 <= 256) -> 4 banks, sharing ONE pool with the
        # projection tiles (2 x 2 banks) so aggregation interleaves with
        # projection without bank collisions.
        pop = ctx.enter_context(tc.tile_pool(name="po", bufs=1, space="PSUM"))
        po_all = pop.tile([P, 4 * BANK], F32, tag="poall")
        pouts = [po_all[:, i * 256 : i * 256 + HCOLS] for i in range(NI)]
        # start=True wipes the WHOLE destination bank (data + has_written),
        # so packed accumulators can't each carry their own start. Wipe each
        # bank once with a dummy 1-col matmul; the aggregation then runs
        # start=False throughout (first write per element lands via the
        # cleared has_written bits).
        z16 = persist.tile([P, 1], F16, tag="z16")
        nc.vector.memset(z16[:], 0.0)
        for bk in range(4):
            nc.tensor.matmul(
                po_all[:, bk * BANK : bk * BANK + 1],
                f1rep[:, 0:P],
                z16[:],
                start=True,
                stop=True,
            )

        group_q0 = []
        q0 = 0
        for gsz in GROUPS:
            group_q0.append(q0)
            q0 += gsz

        deferred = []  # groups produced but not yet masked/aggregated

        def emit_group_front(g):
            """Produce w = tanh(LAM*s + MU) for the group: either DVE
            preadds + one fused tanh, or per-chunk ACT-bias tanh (f2 rides
            the per-partition bias, no preadd)."""
            gsz = GROUPS[g]
            q0 = group_q0[g]
            s_sup = sup.tile([P, gsz * ROWS], F16, tag="s", bufs=3, name=f"s{g}")
            at = adjp.tile(
                [P, gsz * ROWS], F16, tag="at", bufs=3, name=f"at{g}"
            )
            at3 = at[:].rearrange("p (q i) -> p q i", i=ROWS)
            for qq in range(gsz):
                nc.sync.dma_start(
                    at3[:, qq : qq + 1, :], adjs[:, q0 + qq : q0 + qq + 1, :]
                )
            if g in BIAS_GROUPS:
                for qq in range(gsz):
                    q = q0 + qq
                    nc.scalar.activation(
                        s_sup[:, qq * ROWS : (qq + 1) * ROWS],
                        f1rep[:],
                        AF.Tanh,
                        bias=f2h_all[:, q : q + 1],
                    )
            else:
                for qq in range(gsz):
                    q = q0 + qq
                    nc.vector.tensor_scalar_add(
                        s_sup[:, qq * ROWS : (qq + 1) * ROWS],
                        f1rep[:],
                        f2h_all[:, q : q + 1],
                    )
                nc.scalar.activation(s_sup[:], s_sup[:], AF.Tanh, bias=zero1[:])
            return {"g": g, "gsz": gsz, "q0": q0, "w": s_sup, "at": at}

        def emit_group_back(fr):
            """grouped +C0 then mask multiply (both in place on the group
            tile; the fp8 adj auto-converts), then aggregate matmuls."""
            gsz, q0, w_sup = fr["gsz"], fr["q0"], fr["w"]
            et = etp.tile(
                [P, gsz * ROWS], F16, tag="et", bufs=2, name=f"et{q0}"
            )
            nc.vector.tensor_scalar_add(w_sup[:], w_sup[:], float(C0))
            for qq in range(gsz):
                sl = slice(qq * ROWS, (qq + 1) * ROWS)
                nc.vector.tensor_mul(et[:, sl], w_sup[:, sl], fr["at"][:, sl])
            for qq in range(gsz):
                q = q0 + qq
                rhs = h16_all[:, q * HCOLS : (q + 1) * HCOLS]
                # consecutive matmuls must hit different PSUM banks (same-
                # bank accumulation serializes the PE): walk banks 0123 0123
                for it in (0, 2, 4, 6, 1, 3, 5, 7):
                    nc.tensor.matmul(
                        pouts[it],
                        et[:, qq * ROWS + it * P : qq * ROWS + (it + 1) * P],
                        rhs,
                        start=False,
                        stop=(q == NT - 1),
                    )

        # ---- h-projection in mini-batches of 2 node tiles (double-buffered
        # 2-bank PSUM tiles), with group fronts/backs interleaved so the
        # tanh chain, mask ops and aggregation all overlap projection ----
        next_group = 0
        hc = C_OUT if b_zero else HCOLS
        # groups covered by the f2 head start don't wait for projection
        while (
            next_group < len(GROUPS)
            and group_q0[next_group] + GROUPS[next_group] <= F2HEAD
        ):
            deferred.append(emit_group_front(next_group))
            next_group += 1
        # bulk xt1 slices AFTER the head-start fronts so the first adj
        # group tiles don't queue behind 4MB of features
        for c in range(1, len(SUBS) - 1):
            for k in range(nkc):
                if KC[k] != P:
                    if c == 1:
                        nc.sync.dma_start(
                            xts[k][:], xt1[offs[k] : offs[k] + KC[k], :]
                        )
                    continue
                nc.sync.dma_start(
                    xts[k][:, SUBS[c] : SUBS[c + 1]],
                    xt1[offs[k] : offs[k] + KC[k], SUBS[c] : SUBS[c + 1]],
                )
        if True:
            for mb in range(NT // 2):  # mini-batches of 2 node tiles
                ph = pop.tile(
                    [P, 2 * BANK], F32, tag="ph", bufs=2, name=f"ph{mb}"
                )
                nt0 = 2 * mb
                for k in range(nkc):
                    nc.tensor.matmul(
                        ph[:, 0:WCOLS],
                        xts[k][:, nt0 * P : (nt0 + 1) * P],
                        wes[k][:],
                        start=(k == 0),
                        stop=(k == nkc - 1),
                    )
                    nc.tensor.matmul(
                        ph[:, BANK : BANK + WCOLS],
                        xts[k][:, (nt0 + 1) * P : (nt0 + 2) * P],
                        wes[k][:],
                        start=(k == 0),
                        stop=(k == nkc - 1),
                    )
                # drain h (+f2 col) of the 2 fresh tiles
                src = ph[:].rearrange("p (b w) -> p b w", b=2)
                dst_h = h16_all[:, nt0 * HCOLS : (nt0 + 2) * HCOLS].rearrange(
                    "p (b w) -> p b w", b=2
                )
                nc.vector.tensor_copy(dst_h[:, :, 0:hc], src[:, :, 0:hc])
                if nt0 >= 8:  # first 8 f2 columns came from the head start
                    nc.vector.tensor_copy(
                        f2h_all[:, nt0 : nt0 + 2],
                        src[:, :, C_OUT + 2 : C_OUT + 3],
                    )
                done = 2 * (mb + 1)  # chunks fully drained
                while (
                    next_group < len(GROUPS)
                    and group_q0[next_group] + GROUPS[next_group] <= done
                    and len(deferred) < 3
                ):
                    deferred.append(emit_group_front(next_group))
                    next_group += 1

        # ---- flush remaining groups ----
        while deferred or next_group < len(GROUPS):
            if next_group < len(GROUPS) and len(deferred) < 3:
                deferred.append(emit_group_front(next_group))
                next_group += 1
            if deferred:
                emit_group_back(deferred.pop(0))

        # ---- epilogue: divide by clamped denominator, one batched store ----
        ob_all = obp.tile([P, NI * C_OUT], F32, tag="oball")
        po3 = po_all[:].rearrange("p (t w) -> p t w", w=256)
        dm = obp.tile([P, NI], F32, tag="dm")
        nc.vector.tensor_scalar_max(
            dm[:], po3[:, :, C_OUT : C_OUT + 1], TINY
        )
        rc = obp.tile([P, NI], F32, tag="rc")
        nc.vector.reciprocal(rc[:], dm[:])
        for it in range(NI):
            # alternate engines: ACT is idle after the last tanh
            if it % 2 == 0:
                nc.vector.tensor_scalar_mul(
                    ob_all[:, it * C_OUT : (it + 1) * C_OUT],
                    po_all[:, it * 256 : it * 256 + C_OUT],
                    rc[:, it : it + 1],
                )
            else:
                nc.scalar.mul(
                    ob_all[:, it * C_OUT : (it + 1) * C_OUT],
                    po_all[:, it * 256 : it * 256 + C_OUT],
                    rc[:, it : it + 1],
                )
        outr = out.rearrange("(t p) c -> p t c", p=P)
        obr = ob_all[:].rearrange("p (t c) -> p t c", c=C_OUT)
        for tp in range(4):
            nc.sync.dma_start(
                outr[:, 2 * tp : 2 * (tp + 1), :],
                obr[:, 2 * tp : 2 * (tp + 1), :],
            )


def _prep_inputs(node_feats, adj_matrix, W, b, v0, v1):
    X = np.ascontiguousarray(node_feats, dtype=np.float32)
    W = np.asarray(W, dtype=np.float32)
    b = np.asarray(b, dtype=np.float32)
    v0 = np.asarray(v0, dtype=np.float32)
    v1 = np.asarray(v1, dtype=np.float32)

    w0l = (LAM * (W.astype(np.float64) @ v0.astype(np.float64))).astype(np.float32)
    w1l = (LAM * (W.astype(np.float64) @ v1.astype(np.float64))).astype(np.float32)
    c0l = np.float32(LAM * float(b.astype(np.float64) @ v0.astype(np.float64)))
    c1l = np.float32(LAM * float(b.astype(np.float64) @ v1.astype(np.float64)))

    XT1 = np.empty((257, N), np.float32)
    XT1[:256] = X.T
    XT1[256] = 1.0

    WE = np.zeros((257, WCOLS), np.float32)
    WE[:256, :C_OUT] = W
    WE[256, :C_OUT] = b
    WE[256, C_OUT] = 1.0          # makes h_ext column 128 identically 1
    WE[:256, C_OUT + 1] = w0l
    WE[256, C_OUT + 1] = c0l
    WE[:256, C_OUT + 2] = w1l
    WE[256, C_OUT + 2] = c1l

    XT1h = XT1.astype(np.float16)
    WEh = WE.astype(np.float16)
    A16 = np.asarray(adj_matrix, dtype=np.float16)

    in_maps = []
    for c in range(NCORES):
        in_maps.append(
            {
                "xt1": XT1h,
                "xt1l": np.ascontiguousarray(XT1h[:, c * ROWS : (c + 1) * ROWS]),
                "wext": WEh,
                "adjt": np.ascontiguousarray(
                    A16[c * ROWS : (c + 1) * ROWS, :].T
                ),
            }
        )
    return in_maps


def _run(in_maps, trace=False, b_zero=True):
    key = f"nc_b{int(b_zero)}"
    if key not in _CACHE:
        _CACHE[key] = _build_nc(b_zero=b_zero)
    nc = _CACHE[key]
    res = run_bass_kernel_spmd(
        nc, in_maps, core_ids=list(range(NCORES)), trace=trace
    )
    full = np.concatenate(
        [res.results[c]["out"] for c in range(NCORES)], axis=0
    ).astype(np.float32)
    return full, res


def kernel(node_feats, adj_matrix, W, b, v0, v1):
    in_maps = _prep_inputs(node_feats, adj_matrix, W, b, v0, v1)
    trace = bool(int(os.environ.get("GAT_TRACE", "0")))
    b_zero = not bool(np.any(np.asarray(b)))
    full, _ = _run(in_maps, trace=trace, b_zero=b_zero)
    return full


# revision 35
# speedup vs baseline: 1.0161x; 1.0111x over previous
"""GAT single-head forward on 8 Trainium2 NeuronCores (Bass/Tile).

Math (per reference):
    h   = X @ W + b                      [N, 128]
    f1  = h @ v0, f2 = h @ v1            [N]
    logits = adj * (f1[:,None] + f2[None,:])   (adj entries are exactly 0/1)
    vals = sigmoid(logits) - 0.5
    masked softmax over row edges; out = probs @ h

Key identities used on device:
  * On edges (adj==1) the softmax weight is phi(s) = exp(sigmoid(s)) with
    s = f1_i + f2_j (constant shifts cancel; exp never overflows).
  * phi(s) ~= A + B*tanh(LAM*s + MU) to 5.7e-4 max relative error over the
    full attainable s range. The softmax ratio cancels the global factor B,
    so on device only  et = (tanh(LAM*s+MU) + A/B) * adj  is needed:
        probs = et / rowsum(et).
    This replaces the baseline's tanh+exp double activation pass (the 143us
    kernel's bottleneck: ACT busy 87%) with a SINGLE tanh pass.
  * The tanh argument is built for free by the ACT unit itself: the input is
    f1 (LAM-prescaled, MU-shifted) broadcast across partitions, and LAM*f2
    rides the per-partition activation bias, one [P,1] column per j-chunk.
    No separate dense pre-add pass exists at all.
  * The +C0 shift is one grouped in-place tensor_scalar per group (~0.29
    ns/col); the mask is a per-chunk 1024-wide tensor_tensor multiply
    (~0.67 ns/col). Measured on HW: a fused scalar_tensor_tensor runs ~1.6x
    slower than TT, 8192-wide TT runs ~1.6x slower than 1024-wide, and an
    fp8 second operand slows DVE in-situ - hence fp16 adj, narrow TTs, and
    the TS+TT split.
  * A ones-column appended to h turns the softmax denominator into one extra
    matmul output column.

Sharding: rows of adj across the 8 cores (1024 rows each). node_feats is
small and replicated; every core computes the full projected h locally -
no collectives.

Per-core layout: adj block transposed ([j=source node on partitions, i=own
rows on free dim]) so the aggregate probs@h contracts over the partition
dim. adj streams as fp16 group tiles (triple-buffered) whose DMAs are
emitted inside each group front; the first fronts are emitted before the
bulk feature loads so their adj tiles don't queue behind 4MB of xt1.

PSUM: the 8 output accumulators pack two per bank (129 cols at offsets
0/256) in 4 banks, coexisting with the 2x2-bank projection pool. A matmul
with start=True wipes the WHOLE destination bank (verified on HW), so the
banks are wiped once by dummy 1-col matmuls and all aggregation matmuls run
start=False, landing via the cleared has_written bits. Consecutive
aggregation matmuls walk banks 0123 0123 (same-bank accumulation would
serialize the PE).

Schedule shape (engines are in-order; emission order seeds the queues):
  preamble -> [weights/features DMA | f1 path | f2 head] -> group fronts
  (adj DMA + 8 bias-trick tanhs each) pipelined 3 deep against group backs
  (grouped +C0, per-chunk mask TTs, 8 aggregation matmuls per chunk), with
  the h-projection mini-batches interleaved ahead -> epilogue divide and a
  4-way-split output store.
"""

import os

import numpy as np
import ml_dtypes

import concourse.mybir as mybir
import concourse.tile as tile
from concourse import bacc
from concourse.bass_utils import run_bass_kernel_spmd

F32 = mybir.dt.float32
F16 = mybir.dt.float16
BF16 = mybir.dt.bfloat16
F8 = mybir.dt.float8e4
AF = mybir.ActivationFunctionType
ALU = mybir.AluOpType

N, C_IN, C_OUT = 8192, 256, 128
NCORES = 8
ROWS = N // NCORES          # 1024 rows of adj per core
P = 128
NT = N // P                 # 64 node tiles (also the j-chunks)
NI = ROWS // P              # 8 output row-tiles per core
KC = [128, 128, 1]          # contraction chunks of K=257 (X.T rows + ones row)
WCOLS = C_OUT + 3           # [W | ones-hack | LAM*w0 | LAM*w1]
HCOLS = C_OUT + 1           # h plus the ones column
TINY = float(np.finfo(np.float32).tiny)
BANK = 512                  # PSUM bank, fp32 elements

# phi(s) = exp(sigmoid(s)) ~= A + B*tanh(LAM*s + MU); only C0 = A/B survives
# the softmax normalization.
LAM = 0.5082714
MU = -0.24995726
C0 = 1.85905591 / 0.85894722

# activation groups: j-chunks per pipeline stage. Small leading groups start
# the ACT chain early; small trailing groups shorten the tail.
GROUPS = [2, 4] + [8] * 6 + [4, 4, 2]
# groups whose chunks compute tanh(f1rep + f2) directly via the ACT
# per-partition bias (no DVE preadd, ~+155ns/chunk on ACT): balances the
# in-order DVE queue (preadds + grouped add/mask) against ACT.
BIAS_GROUPS = set(range(16))

_CACHE: dict = {}


def _build_nc(b_zero=True):
    nc = bacc.Bacc(
        "TRN2", target_bir_lowering=False, debug=False, num_devices=NCORES
    )
    xt1 = nc.dram_tensor("xt1", [257, N], F16, kind="ExternalInput").ap()
    xt1l = nc.dram_tensor("xt1l", [257, ROWS], F16, kind="ExternalInput").ap()
    wext = nc.dram_tensor("wext", [257, WCOLS], F16, kind="ExternalInput").ap()
    adjt = nc.dram_tensor("adjt", [N, ROWS], F16, kind="ExternalInput").ap()
    out = nc.dram_tensor("out", [ROWS, C_OUT], F32, kind="ExternalOutput").ap()

    with tile.TileContext(nc) as tc:
        _emit(tc, nc, xt1, xt1l, wext, adjt, out, b_zero)
    nc.compile()
    return nc


def _emit(tc, nc, xt1, xt1l, wext, adjt, out, b_zero):
    from contextlib import ExitStack

    # with b == 0 the K=1 "ones row" contraction chunk only contributes the
    # constant-one column of h_ext (done with a strided memset instead) and
    # zero constants to f1/f2 -- skip it entirely.
    nkc = 2 if b_zero else 3

    with ExitStack() as ctx:
        # ---- persistent tiles ----
        persist = ctx.enter_context(tc.tile_pool(name="persist", bufs=1))
        h16_all = persist.tile([P, NT * HCOLS], F16, tag="h16")   # [128, 8256]
        f2h_all = persist.tile([P, NT], F32, tag="f2h")           # LAM*f2 per j
        f1rep = persist.tile([P, ROWS], F16, tag="f1rep")         # LAM*f1+MU
        zero1 = persist.tile([P, 1], F32, tag="zero1")
        nc.vector.memset(zero1[:], 0.0)
        mu1 = persist.tile([P, 1], F32, tag="mu1")
        nc.vector.memset(mu1[:], MU)
        if b_zero:
            # constant-one column of every h_ext tile (replaces the K=1
            # bias matmul chunk)
            nc.vector.memset(
                h16_all[:].rearrange("p (t c) -> p t c", c=HCOLS)[
                    :, :, C_OUT : C_OUT + 1
                ],
                1.0,
            )

        xtp = ctx.enter_context(tc.tile_pool(name="xt", bufs=1))

        # ---- input loads ----
        # small inputs first so the f1 path clears quickly; adj chunk DMAs
        # round-robin across queues, interleaved with the xt1 column slices
        # so early chunks land before their mask-STT needs them.
        offs = [0, 128, 256]
        xts = [
            xtp.tile([KC[k], N], F16, name=f"xtsb{k}", tag=f"xt{k}")
            for k in range(nkc)
        ]
        SUBS = [0, 1024, 3072, 5120, N]
        wes, xls = [], []
        off = 0
        for k in range(nkc):
            kc = KC[k]
            wx_sb = xtp.tile([kc, WCOLS + ROWS], F16, name=f"wx{k}", tag=f"wx{k}")
            nc.sync.dma_start(wx_sb[:, 0:WCOLS], wext[off : off + kc, :])
            nc.sync.dma_start(wx_sb[:, WCOLS:], xt1l[off : off + kc, :])
            wes.append(wx_sb[:, 0:WCOLS])
            xls.append(wx_sb[:, WCOLS:])
            off += kc
        for k in range(nkc):
            if KC[k] == P:
                nc.sync.dma_start(
                    xts[k][:, 0 : SUBS[1]],
                    xt1[offs[k] : offs[k] + KC[k], 0 : SUBS[1]],
                )

        adjs = adjt.rearrange("(q p) i -> p q i", p=P)


        # ---- f1 path: LAM*f1 + MU for this core's rows, replicated across
        # all partitions by a matmul whose stationary operand is the LAM*w0
        # column broadcast across the 128 PE columns ----
        with tc.tile_pool(name="pf", bufs=1, space="PSUM") as pfp:
            prep = pfp.tile([P, ROWS], F32, tag="prep")
            for k in range(nkc):
                for nh in range(ROWS // 512):
                    nc.tensor.matmul(
                        prep[:, nh * 512 : (nh + 1) * 512],
                        wes[k][:, C_OUT + 1 : C_OUT + 2].to_broadcast(
                            (KC[k], P)
                        ),
                        xls[k][:, nh * 512 : (nh + 1) * 512],
                        start=(k == 0),
                        stop=(k == nkc - 1),
                    )
            nc.scalar.activation(
                f1rep[:], prep[:], AF.Identity, bias=mu1[:], scale=1.0
            )

        # ---- f2 head start: LAM*f2 for the first 8 j-chunks via tiny
        # direct matmuls so activation groups 0/1 don't wait for the
        # h-projection pipeline ----
        F2HEAD = 8
        with tc.tile_pool(name="pf2", bufs=1, space="PSUM") as pf2p:
            pt = pf2p.tile([P, NI * BANK], F32, tag="pt")
            pt3 = pt[:].rearrange("p (t w) -> p t w", w=BANK)
            for q in range(F2HEAD):
                w = (q % NI) * BANK
                for k in range(nkc):
                    nc.tensor.matmul(
                        pt[:, w : w + 1],
                        xts[k][:, q * P : (q + 1) * P],
                        wes[k][:, C_OUT + 2 : C_OUT + 3],
                        start=(k == 0),
                        stop=(k == nkc - 1),
                    )
                if q == 1:
                    # group 0's two columns drain immediately so its
                    # preadds (and the whole activation chain) start early
                    nc.vector.tensor_copy(
                        f2h_all[:, 0:2], pt3[:, 0:2, 0:1]
                    )
            nc.vector.tensor_copy(
                f2h_all[:, 2:F2HEAD], pt3[:, 2:F2HEAD, 0:1]
            )

        # ---- main-loop pools ----
        sup = ctx.enter_context(tc.tile_pool(name="sup", bufs=1))
        adjp = ctx.enter_context(tc.tile_pool(name="adjp", bufs=1))
        etp = ctx.enter_context(tc.tile_pool(name="etp", bufs=1))
        obp = ctx.enter_context(tc.tile_pool(name="ob", bufs=2))

        # aggregate accumulators: 8 row-tiles x 129 cols packed two per
        # PSUM bank (129 <= 256) -> 4 banks, sharing ONE pool with the
        # projection tiles (2 x 2 banks) so aggregation interleaves with
        # projection without bank collisions.
        pop = ctx.enter_context(tc.tile_pool(name="po", bufs=1, space="PSUM"))
        po_all = pop.tile([P, 4 * BANK], F32, tag="poall")
        pouts = [po_all[:, i * 256 : i * 256 + HCOLS] for i in range(NI)]
        # start=True wipes the WHOLE destination bank (data + has_written),
        # so packed accumulators can't each carry their own start. Wipe each
        # bank once with a dummy 1-col matmul; the aggregation then runs
        # start=False throughout (first write per element lands via the
        # cleared has_written bits).
        z16 = persist.tile([P, 1], F16, tag="z16")
        nc.vector.memset(z16[:], 0.0)
        for bk in range(4):
            nc.tensor.matmul(
                po_all[:, bk * BANK : bk * BANK + 1],
                f1rep[:, 0:P],
                z16[:],
                start=True,
                stop=True,
            )

        group_q0 = []
        q0 = 0
        for gsz in GROUPS:
            group_q0.append(q0)
            q0 += gsz

        deferred = []  # groups produced but not yet masked/aggregated

        def emit_group_front(g):
            """Produce w = tanh(LAM*s + MU) for the group: either DVE
            preadds + one fused tanh, or per-chunk ACT-bias tanh (f2 rides
            the per-partition bias, no preadd)."""
            gsz = GROUPS[g]
            q0 = group_q0[g]
            s_sup = sup.tile([P, gsz * ROWS], F16, tag="s", bufs=3, name=f"s{g}")
            at = adjp.tile(
                [P, gsz * ROWS], F16, tag="at", bufs=3, name=f"at{g}"
            )
            at3 = at[:].rearrange("p (q i) -> p q i", i=ROWS)
            for qq in range(gsz):
                nc.sync.dma_start(
                    at3[:, qq : qq + 1, :], adjs[:, q0 + qq : q0 + qq + 1, :]
                )
            if g in BIAS_GROUPS:
                for qq in range(gsz):
                    q = q0 + qq
                    nc.scalar.activation(
                        s_sup[:, qq * ROWS : (qq + 1) * ROWS],
                        f1rep[:],
                        AF.Tanh,
                        bias=f2h_all[:, q : q + 1],
                    )
            else:
                for qq in range(gsz):
                    q = q0 + qq
                    nc.vector.tensor_scalar_add(
                        s_sup[:, qq * ROWS : (qq + 1) * ROWS],
                        f1rep[:],
                        f2h_all[:, q : q + 1],
                    )
                nc.scalar.activation(s_sup[:], s_sup[:], AF.Tanh, bias=zero1[:])
            return {"g": g, "gsz": gsz, "q0": q0, "w": s_sup, "at": at}

        def emit_group_back(fr):
            """grouped +C0 then mask multiply (both in place on the group
            tile; the fp8 adj auto-converts), then aggregate matmuls."""
            gsz, q0, w_sup = fr["gsz"], fr["q0"], fr["w"]
            et = etp.tile(
                [P, gsz * ROWS], F16, tag="et", bufs=2, name=f"et{q0}"
            )
            nc.vector.tensor_scalar_add(w_sup[:], w_sup[:], float(C0))
            for qq in range(gsz):
                sl = slice(qq * ROWS, (qq + 1) * ROWS)
                nc.vector.tensor_mul(et[:, sl], w_sup[:, sl], fr["at"][:, sl])
            for qq in range(gsz):
                q = q0 + qq
                rhs = h16_all[:, q * HCOLS : (q + 1) * HCOLS]
                # consecutive matmuls must hit different PSUM banks (same-
                # bank accumulation serializes the PE): walk banks 0123 0123
                for it in (0, 2, 4, 6, 1, 3, 5, 7):
                    nc.tensor.matmul(
                        pouts[it],
                        et[:, qq * ROWS + it * P : qq * ROWS + (it + 1) * P],
                        rhs,
                        start=False,
                        stop=(q == NT - 1),
                    )

        # ---- h-projection in mini-batches of 2 node tiles (double-buffered
        # 2-bank PSUM tiles), with group fronts/backs interleaved so the
        # tanh chain, mask ops and aggregation all overlap projection ----
        next_group = 0
        hc = C_OUT if b_zero else HCOLS
        # groups covered by the f2 head start don't wait for projection
        while (
            next_group < len(GROUPS)
            and group_q0[next_group] + GROUPS[next_group] <= F2HEAD
        ):
            deferred.append(emit_group_front(next_group))
            next_group += 1
        # bulk xt1 slices AFTER the head-start fronts so the first adj
        # group tiles don't queue behind 4MB of features
        for c in range(1, len(SUBS) - 1):
            for k in range(nkc):
                if KC[k] != P:
                    if c == 1:
                        nc.sync.dma_start(
                            xts[k][:], xt1[offs[k] : offs[k] + KC[k], :]
                        )
                    continue
                nc.sync.dma_start(
                    xts[k][:, SUBS[c] : SUBS[c + 1]],
                    xt1[offs[k] : offs[k] + KC[k], SUBS[c] : SUBS[c + 1]],
                )
        if True:
            for mb in range(NT // 2):  # mini-batches of 2 node tiles
                ph = pop.tile(
                    [P, 2 * BANK], F32, tag="ph", bufs=2, name=f"ph{mb}"
                )
                nt0 = 2 * mb
                for k in range(nkc):
                    nc.tensor.matmul(
                        ph[:, 0:WCOLS],
                        xts[k][:, nt0 * P : (nt0 + 1) * P],
                        wes[k][:],
                        start=(k == 0),
                        stop=(k == nkc - 1),
                    )
                    nc.tensor.matmul(
                        ph[:, BANK : BANK + WCOLS],
                        xts[k][:, (nt0 + 1) * P : (nt0 + 2) * P],
                        wes[k][:],
                        start=(k == 0),
                        stop=(k == nkc - 1),
                    )
                # drain h (+f2 col) of the 2 fresh tiles
                src = ph[:].rearrange("p (b w) -> p b w", b=2)
                dst_h = h16_all[:, nt0 * HCOLS : (nt0 + 2) * HCOLS].rearrange(
                    "p (b w) -> p b w", b=2
                )
                nc.vector.tensor_copy(dst_h[:, :, 0:hc], src[:, :, 0:hc])
                if nt0 >= 8:  # first 8 f2 columns came from the head start
                    nc.vector.tensor_copy(
                        f2h_all[:, nt0 : nt0 + 2],
                        src[:, :, C_OUT + 2 : C_OUT + 3],
                    )
                done = 2 * (mb + 1)  # chunks fully drained
                while (
                    next_group < len(GROUPS)
                    and group_q0[next_group] + GROUPS[next_group] <= done
                    and len(deferred) < 3
                ):
                    deferred.append(emit_group_front(next_group))
                    next_group += 1

        # ---- flush remaining groups ----
        while deferred or next_group < len(GROUPS):
            if next_group < len(GROUPS) and len(deferred) < 3:
                deferred.append(emit_group_front(next_group))
                next_group += 1
            if deferred:
                emit_group_back(deferred.pop(0))

        # ---- epilogue: divide by clamped denominator, one batched store ----
        ob_all = obp.tile([P, NI * C_OUT], F32, tag="oball")
        po3 = po_all[:].rearrange("p (t w) -> p t w", w=256)
        dm = obp.tile([P, NI], F32, tag="dm")
        nc.vector.tensor_scalar_max(
            dm[:], po3[:, :, C_OUT : C_OUT + 1], TINY
        )
        rc = obp.tile([P, NI], F32, tag="rc")
        nc.vector.reciprocal(rc[:], dm[:])
        for it in range(NI):
            # alternate engines: ACT is idle after the last tanh
            if it % 2 == 0:
                nc.vector.tensor_scalar_mul(
                    ob_all[:, it * C_OUT : (it + 1) * C_OUT],
                    po_all[:, it * 256 : it * 256 + C_OUT],
                    rc[:, it : it + 1],
                )
            else:
                nc.scalar.mul(
                    ob_all[:, it * C_OUT : (it + 1) * C_OUT],
                    po_all[:, it * 256 : it * 256 + C_OUT],
                    rc[:, it : it + 1],
                )
        outr = out.rearrange("(t p) c -> p t c", p=P)
        obr = ob_all[:].rearrange("p (t c) -> p t c", c=C_OUT)
        for tp in range(4):
            nc.sync.dma_start(
                outr[:, 2 * tp : 2 * (tp + 1), :],
                obr[:, 2 * tp : 2 * (tp + 1), :],
            )


def _prep_inputs(node_feats, adj_matrix, W, b, v0, v1):
    X = np.ascontiguousarray(node_feats, dtype=np.float32)
    W = np.asarray(W, dtype=np.float32)
    b = np.asarray(b, dtype=np.float32)
    v0 = np.asarray(v0, dtype=np.float32)
    v1 = np.asarray(v1, dtype=np.float32)

    w0l = (LAM * (W.astype(np.float64) @ v0.astype(np.float64))).astype(np.float32)
    w1l = (LAM * (W.astype(np.float64) @ v1.astype(np.float64))).astype(np.float32)
    c0l = np.float32(LAM * float(b.astype(np.float64) @ v0.astype(np.float64)))
    c1l = np.float32(LAM * float(b.astype(np.float64) @ v1.astype(np.float64)))

    XT1 = np.empty((257, N), np.float32)
    XT1[:256] = X.T
    XT1[256] = 1.0

    WE = np.zeros((257, WCOLS), np.float32)
    WE[:256, :C_OUT] = W
    WE[256, :C_OUT] = b
    WE[256, C_OUT] = 1.0          # makes h_ext column 128 identically 1
    WE[:256, C_OUT + 1] = w0l
    WE[256, C_OUT + 1] = c0l
    WE[:256, C_OUT + 2] = w1l
    WE[256, C_OUT + 2] = c1l

    XT1h = XT1.astype(np.float16)
    WEh = WE.astype(np.float16)
    A16 = np.asarray(adj_matrix, dtype=np.float16)

    in_maps = []
    for c in range(NCORES):
        in_maps.append(
            {
                "xt1": XT1h,
                "xt1l": np.ascontiguousarray(XT1h[:, c * ROWS : (c + 1) * ROWS]),
                "wext": WEh,
                "adjt": np.ascontiguousarray(
                    A16[c * ROWS : (c + 1) * ROWS, :].T
                ),
            }
        )
    return in_maps


def _run(in_maps, trace=False, b_zero=True):
    key = f"nc_b{int(b_zero)}"
    if key not in _CACHE:
        _CACHE[key] = _build_nc(b_zero=b_zero)
    nc = _CACHE[key]
    res = run_bass_kernel_spmd(
        nc, in_maps, core_ids=list(range(NCORES)), trace=trace
    )
    full = np.concatenate(
        [res.results[c]["out"] for c in range(NCORES)], axis=0
    ).astype(np.float32)
    return full, res


def kernel(node_feats, adj_matrix, W, b, v0, v1):
    in_maps = _prep_inputs(node_feats, adj_matrix, W, b, v0, v1)
    trace = bool(int(os.environ.get("GAT_TRACE", "0")))
    b_zero = not bool(np.any(np.asarray(b)))
    full, _ = _run(in_maps, trace=trace, b_zero=b_zero)
    return full


# revision 36
# speedup vs baseline: 1.0171x; 1.0010x over previous
"""GAT single-head forward on 8 Trainium2 NeuronCores (Bass/Tile).

Math (per reference):
    h   = X @ W + b                      [N, 128]
    f1  = h @ v0, f2 = h @ v1            [N]
    logits = adj * (f1[:,None] + f2[None,:])   (adj entries are exactly 0/1)
    vals = sigmoid(logits) - 0.5
    masked softmax over row edges; out = probs @ h

Key identities used on device:
  * On edges (adj==1) the softmax weight is phi(s) = exp(sigmoid(s)) with
    s = f1_i + f2_j (constant shifts cancel; exp never overflows).
  * phi(s) ~= A + B*tanh(LAM*s + MU) to 5.7e-4 max relative error over the
    full attainable s range. The softmax ratio cancels the global factor B,
    so on device only  et = (tanh(LAM*s+MU) + A/B) * adj  is needed:
        probs = et / rowsum(et).
    This replaces the baseline's tanh+exp double activation pass (the 143us
    kernel's bottleneck: ACT busy 87%) with a SINGLE tanh pass.
  * The tanh argument is built for free by the ACT unit itself: the input is
    f1 (LAM-prescaled, MU-shifted) broadcast across partitions, and LAM*f2
    rides the per-partition activation bias, one [P,1] column per j-chunk.
    No separate dense pre-add pass exists at all.
  * The +C0 shift is one grouped in-place tensor_scalar per group (~0.29
    ns/col); the mask is a per-chunk 1024-wide tensor_tensor multiply
    (~0.67 ns/col). Measured on HW: a fused scalar_tensor_tensor runs ~1.6x
    slower than TT, 8192-wide TT runs ~1.6x slower than 1024-wide, and an
    fp8 second operand slows DVE in-situ - hence fp16 adj, narrow TTs, and
    the TS+TT split.
  * A ones-column appended to h turns the softmax denominator into one extra
    matmul output column.

Sharding: rows of adj across the 8 cores (1024 rows each). node_feats is
small and replicated; every core computes the full projected h locally -
no collectives.

Per-core layout: adj block transposed ([j=source node on partitions, i=own
rows on free dim]) so the aggregate probs@h contracts over the partition
dim. adj streams as fp16 group tiles (triple-buffered) whose DMAs are
emitted inside each group front; the first fronts are emitted before the
bulk feature loads so their adj tiles don't queue behind 4MB of xt1.

PSUM: the 8 output accumulators pack two per bank (129 cols at offsets
0/256) in 4 banks, coexisting with the 2x2-bank projection pool. A matmul
with start=True wipes the WHOLE destination bank (verified on HW), so the
banks are wiped once by dummy 1-col matmuls and all aggregation matmuls run
start=False, landing via the cleared has_written bits. Consecutive
aggregation matmuls walk banks 0123 0123 (same-bank accumulation would
serialize the PE).

Schedule shape (engines are in-order; emission order seeds the queues):
  preamble -> [weights/features DMA | f1 path | f2 head] -> group fronts
  (adj DMA + 8 bias-trick tanhs each) pipelined 3 deep against group backs
  (grouped +C0, per-chunk mask TTs, 8 aggregation matmuls per chunk), with
  the h-projection mini-batches interleaved ahead -> epilogue divide and a
  4-way-split output store.
"""

import os

import numpy as np
import ml_dtypes

import concourse.mybir as mybir
import concourse.tile as tile
from concourse import bacc
from concourse.bass_utils import run_bass_kernel_spmd

F32 = mybir.dt.float32
F16 = mybir.dt.float16
BF16 = mybir.dt.bfloat16
F8 = mybir.dt.float8e4
AF = mybir.ActivationFunctionType
ALU = mybir.AluOpType

N, C_IN, C_OUT = 8192, 256, 128
NCORES = 8
ROWS = N // NCORES          # 1024 rows of adj per core
P = 128
NT = N // P                 # 64 node tiles (also the j-chunks)
NI = ROWS // P              # 8 output row-tiles per core
KC = [128, 128, 1]          # contraction chunks of K=257 (X.T rows + ones row)
WCOLS = C_OUT + 3           # [W | ones-hack | LAM*w0 | LAM*w1]
HCOLS = C_OUT + 1           # h plus the ones column
TINY = float(np.finfo(np.float32).tiny)
BANK = 512                  # PSUM bank, fp32 elements

# phi(s) = exp(sigmoid(s)) ~= A + B*tanh(LAM*s + MU); only C0 = A/B survives
# the softmax normalization.
LAM = 0.5082714
MU = -0.24995726
C0 = 1.85905591 / 0.85894722

# activation groups: j-chunks per pipeline stage. Small leading groups start
# the ACT chain early; small trailing groups shorten the tail.
GROUPS = [2, 4] + [8] * 6 + [4, 4, 2]
# groups whose chunks compute tanh(f1rep + f2) directly via the ACT
# per-partition bias (no DVE preadd, ~+155ns/chunk on ACT): balances the
# in-order DVE queue (preadds + grouped add/mask) against ACT.
BIAS_GROUPS = set(range(16))

_CACHE: dict = {}


def _build_nc(b_zero=True):
    nc = bacc.Bacc(
        "TRN2", target_bir_lowering=False, debug=False, num_devices=NCORES
    )
    xt1 = nc.dram_tensor("xt1", [257, N], F16, kind="ExternalInput").ap()
    xt1l = nc.dram_tensor("xt1l", [257, ROWS], F16, kind="ExternalInput").ap()
    wext = nc.dram_tensor("wext", [257, WCOLS], F16, kind="ExternalInput").ap()
    adjt = nc.dram_tensor("adjt", [N, ROWS], F16, kind="ExternalInput").ap()
    out = nc.dram_tensor("out", [ROWS, C_OUT], F32, kind="ExternalOutput").ap()

    with tile.TileContext(nc) as tc:
        _emit(tc, nc, xt1, xt1l, wext, adjt, out, b_zero)
    nc.compile()
    return nc


def _emit(tc, nc, xt1, xt1l, wext, adjt, out, b_zero):
    from contextlib import ExitStack

    # with b == 0 the K=1 "ones row" contraction chunk only contributes the
    # constant-one column of h_ext (done with a strided memset instead) and
    # zero constants to f1/f2 -- skip it entirely.
    nkc = 2 if b_zero else 3

    with ExitStack() as ctx:
        # ---- persistent tiles ----
        persist = ctx.enter_context(tc.tile_pool(name="persist", bufs=1))
        h16_all = persist.tile([P, NT * HCOLS], F16, tag="h16")   # [128, 8256]
        f2h_all = persist.tile([P, NT], F32, tag="f2h")           # LAM*f2 per j
        f1rep = persist.tile([P, ROWS], F16, tag="f1rep")         # LAM*f1+MU
        zero1 = persist.tile([P, 1], F32, tag="zero1")
        nc.vector.memset(zero1[:], 0.0)
        mu1 = persist.tile([P, 1], F32, tag="mu1")
        nc.vector.memset(mu1[:], MU)
        warm1 = persist.tile([P, 1], F16, tag="warm1")
        # load the tanh table set at t~0, off the critical path
        nc.scalar.activation(warm1[:], zero1[:], AF.Tanh, bias=zero1[:])
        if b_zero:
            # constant-one column of every h_ext tile (replaces the K=1
            # bias matmul chunk)
            nc.vector.memset(
                h16_all[:].rearrange("p (t c) -> p t c", c=HCOLS)[
                    :, :, C_OUT : C_OUT + 1
                ],
                1.0,
            )

        xtp = ctx.enter_context(tc.tile_pool(name="xt", bufs=1))

        # ---- input loads ----
        # small inputs first so the f1 path clears quickly; adj chunk DMAs
        # round-robin across queues, interleaved with the xt1 column slices
        # so early chunks land before their mask-STT needs them.
        offs = [0, 128, 256]
        xts = [
            xtp.tile([KC[k], N], F16, name=f"xtsb{k}", tag=f"xt{k}")
            for k in range(nkc)
        ]
        SUBS = [0, 1024, 3072, 5120, N]
        wes, xls = [], []
        off = 0
        for k in range(nkc):
            kc = KC[k]
            wx_sb = xtp.tile([kc, WCOLS + ROWS], F16, name=f"wx{k}", tag=f"wx{k}")
            nc.sync.dma_start(wx_sb[:, 0:WCOLS], wext[off : off + kc, :])
            nc.sync.dma_start(wx_sb[:, WCOLS:], xt1l[off : off + kc, :])
            wes.append(wx_sb[:, 0:WCOLS])
            xls.append(wx_sb[:, WCOLS:])
            off += kc
        for k in range(nkc):
            if KC[k] == P:
                nc.sync.dma_start(
                    xts[k][:, 0 : SUBS[1]],
                    xt1[offs[k] : offs[k] + KC[k], 0 : SUBS[1]],
                )

        adjs = adjt.rearrange("(q p) i -> p q i", p=P)


        # ---- f1 path: LAM*f1 + MU for this core's rows, replicated across
        # all partitions by a matmul whose stationary operand is the LAM*w0
        # column broadcast across the 128 PE columns ----
        with tc.tile_pool(name="pf", bufs=1, space="PSUM") as pfp:
            prep = pfp.tile([P, ROWS], F32, tag="prep")
            for k in range(nkc):
                for nh in range(ROWS // 512):
                    nc.tensor.matmul(
                        prep[:, nh * 512 : (nh + 1) * 512],
                        wes[k][:, C_OUT + 1 : C_OUT + 2].to_broadcast(
                            (KC[k], P)
                        ),
                        xls[k][:, nh * 512 : (nh + 1) * 512],
                        start=(k == 0),
                        stop=(k == nkc - 1),
                    )
            nc.scalar.activation(
                f1rep[:], prep[:], AF.Identity, bias=mu1[:], scale=1.0
            )

        # ---- f2 head start: LAM*f2 for the first 8 j-chunks via tiny
        # direct matmuls so activation groups 0/1 don't wait for the
        # h-projection pipeline ----
        F2HEAD = 8
        with tc.tile_pool(name="pf2", bufs=1, space="PSUM") as pf2p:
            pt = pf2p.tile([P, NI * BANK], F32, tag="pt")
            pt3 = pt[:].rearrange("p (t w) -> p t w", w=BANK)
            for q in range(F2HEAD):
                w = (q % NI) * BANK
                for k in range(nkc):
                    nc.tensor.matmul(
                        pt[:, w : w + 1],
                        xts[k][:, q * P : (q + 1) * P],
                        wes[k][:, C_OUT + 2 : C_OUT + 3],
                        start=(k == 0),
                        stop=(k == nkc - 1),
                    )
                if q == 1:
                    # group 0's two columns drain immediately so its
                    # preadds (and the whole activation chain) start early
                    nc.vector.tensor_copy(
                        f2h_all[:, 0:2], pt3[:, 0:2, 0:1]
                    )
            nc.vector.tensor_copy(
                f2h_all[:, 2:F2HEAD], pt3[:, 2:F2HEAD, 0:1]
            )

        # ---- main-loop pools ----
        sup = ctx.enter_context(tc.tile_pool(name="sup", bufs=1))
        adjp = ctx.enter_context(tc.tile_pool(name="adjp", bufs=1))
        etp = ctx.enter_context(tc.tile_pool(name="etp", bufs=1))
        obp = ctx.enter_context(tc.tile_pool(name="ob", bufs=2))

        # aggregate accumulators: 8 row-tiles x 129 cols packed two per
        # PSUM bank (129 <= 256) -> 4 banks, sharing ONE pool with the
        # projection tiles (2 x 2 banks) so aggregation interleaves with
        # projection without bank collisions.
        pop = ctx.enter_context(tc.tile_pool(name="po", bufs=1, space="PSUM"))
        po_all = pop.tile([P, 4 * BANK], F32, tag="poall")
        pouts = [po_all[:, i * 256 : i * 256 + HCOLS] for i in range(NI)]
        # start=True wipes the WHOLE destination bank (data + has_written),
        # so packed accumulators can't each carry their own start. Wipe each
        # bank once with a dummy 1-col matmul; the aggregation then runs
        # start=False throughout (first write per element lands via the
        # cleared has_written bits).
        z16 = persist.tile([P, 1], F16, tag="z16")
        nc.vector.memset(z16[:], 0.0)
        for bk in range(4):
            nc.tensor.matmul(
                po_all[:, bk * BANK : bk * BANK + 1],
                f1rep[:, 0:P],
                z16[:],
                start=True,
                stop=True,
            )

        group_q0 = []
        q0 = 0
        for gsz in GROUPS:
            group_q0.append(q0)
            q0 += gsz

        deferred = []  # groups produced but not yet masked/aggregated

        def emit_group_front(g):
            """Produce w = tanh(LAM*s + MU) for the group: either DVE
            preadds + one fused tanh, or per-chunk ACT-bias tanh (f2 rides
            the per-partition bias, no preadd)."""
            gsz = GROUPS[g]
            q0 = group_q0[g]
            s_sup = sup.tile([P, gsz * ROWS], F16, tag="s", bufs=3, name=f"s{g}")
            at = adjp.tile(
                [P, gsz * ROWS], F16, tag="at", bufs=3, name=f"at{g}"
            )
            at3 = at[:].rearrange("p (q i) -> p q i", i=ROWS)
            for qq in range(gsz):
                nc.sync.dma_start(
                    at3[:, qq : qq + 1, :], adjs[:, q0 + qq : q0 + qq + 1, :]
                )
            if g in BIAS_GROUPS:
                for qq in range(gsz):
                    q = q0 + qq
                    nc.scalar.activation(
                        s_sup[:, qq * ROWS : (qq + 1) * ROWS],
                        f1rep[:],
                        AF.Tanh,
                        bias=f2h_all[:, q : q + 1],
                    )
            else:
                for qq in range(gsz):
                    q = q0 + qq
                    nc.vector.tensor_scalar_add(
                        s_sup[:, qq * ROWS : (qq + 1) * ROWS],
                        f1rep[:],
                        f2h_all[:, q : q + 1],
                    )
                nc.scalar.activation(s_sup[:], s_sup[:], AF.Tanh, bias=zero1[:])
            return {"g": g, "gsz": gsz, "q0": q0, "w": s_sup, "at": at}

        def emit_group_back(fr):
            """grouped +C0 then mask multiply (both in place on the group
            tile; the fp8 adj auto-converts), then aggregate matmuls."""
            gsz, q0, w_sup = fr["gsz"], fr["q0"], fr["w"]
            et = etp.tile(
                [P, gsz * ROWS], F16, tag="et", bufs=2, name=f"et{q0}"
            )
            nc.vector.tensor_scalar_add(w_sup[:], w_sup[:], float(C0))
            for qq in range(gsz):
                sl = slice(qq * ROWS, (qq + 1) * ROWS)
                nc.vector.tensor_mul(et[:, sl], w_sup[:, sl], fr["at"][:, sl])
            for qq in range(gsz):
                q = q0 + qq
                rhs = h16_all[:, q * HCOLS : (q + 1) * HCOLS]
                # consecutive matmuls must hit different PSUM banks (same-
                # bank accumulation serializes the PE): walk banks 0123 0123
                for it in (0, 2, 4, 6, 1, 3, 5, 7):
                    nc.tensor.matmul(
                        pouts[it],
                        et[:, qq * ROWS + it * P : qq * ROWS + (it + 1) * P],
                        rhs,
                        start=False,
                        stop=(q == NT - 1),
                    )

        # ---- h-projection in mini-batches of 2 node tiles (double-buffered
        # 2-bank PSUM tiles), with group fronts/backs interleaved so the
        # tanh chain, mask ops and aggregation all overlap projection ----
        next_group = 0
        hc = C_OUT if b_zero else HCOLS
        # groups covered by the f2 head start don't wait for projection
        while (
            next_group < len(GROUPS)
            and group_q0[next_group] + GROUPS[next_group] <= F2HEAD
        ):
            deferred.append(emit_group_front(next_group))
            next_group += 1
        # bulk xt1 slices AFTER the head-start fronts so the first adj
        # group tiles don't queue behind 4MB of features
        for c in range(1, len(SUBS) - 1):
            for k in range(nkc):
                if KC[k] != P:
                    if c == 1:
                        nc.sync.dma_start(
                            xts[k][:], xt1[offs[k] : offs[k] + KC[k], :]
                        )
                    continue
                nc.sync.dma_start(
                    xts[k][:, SUBS[c] : SUBS[c + 1]],
                    xt1[offs[k] : offs[k] + KC[k], SUBS[c] : SUBS[c + 1]],
                )
        if True:
            for mb in range(NT // 2):  # mini-batches of 2 node tiles
                ph = pop.tile(
                    [P, 2 * BANK], F32, tag="ph", bufs=2, name=f"ph{mb}"
                )
                nt0 = 2 * mb
                for k in range(nkc):
                    nc.tensor.matmul(
                        ph[:, 0:WCOLS],
                        xts[k][:, nt0 * P : (nt0 + 1) * P],
                        wes[k][:],
                        start=(k == 0),
                        stop=(k == nkc - 1),
                    )
                    nc.tensor.matmul(
                        ph[:, BANK : BANK + WCOLS],
                        xts[k][:, (nt0 + 1) * P : (nt0 + 2) * P],
                        wes[k][:],
                        start=(k == 0),
                        stop=(k == nkc - 1),
                    )
                # drain h (+f2 col) of the 2 fresh tiles
                src = ph[:].rearrange("p (b w) -> p b w", b=2)
                dst_h = h16_all[:, nt0 * HCOLS : (nt0 + 2) * HCOLS].rearrange(
                    "p (b w) -> p b w", b=2
                )
                if mb % 5 == 4:
                    nc.scalar.activation(
                        dst_h[:, :, 0:hc], src[:, :, 0:hc], AF.Identity,
                        bias=zero1[:],
                    )
                else:
                    nc.vector.tensor_copy(dst_h[:, :, 0:hc], src[:, :, 0:hc])
                if nt0 >= 8:  # first 8 f2 columns came from the head start
                    nc.vector.tensor_copy(
                        f2h_all[:, nt0 : nt0 + 2],
                        src[:, :, C_OUT + 2 : C_OUT + 3],
                    )
                done = 2 * (mb + 1)  # chunks fully drained
                while (
                    next_group < len(GROUPS)
                    and group_q0[next_group] + GROUPS[next_group] <= done
                    and len(deferred) < 3
                ):
                    deferred.append(emit_group_front(next_group))
                    next_group += 1

        # ---- flush remaining groups ----
        while deferred or next_group < len(GROUPS):
            if next_group < len(GROUPS) and len(deferred) < 3:
                deferred.append(emit_group_front(next_group))
                next_group += 1
            if deferred:
                emit_group_back(deferred.pop(0))

        # ---- epilogue: divide by clamped denominator, one batched store ----
        ob_all = obp.tile([P, NI * C_OUT], F32, tag="oball")
        po3 = po_all[:].rearrange("p (t w) -> p t w", w=256)
        dm = obp.tile([P, NI], F32, tag="dm")
        nc.vector.tensor_scalar_max(
            dm[:], po3[:, :, C_OUT : C_OUT + 1], TINY
        )
        rc = obp.tile([P, NI], F32, tag="rc")
        nc.vector.reciprocal(rc[:], dm[:])
        for it in range(NI):
            # alternate engines: ACT is idle after the last tanh
            if it % 2 == 0:
                nc.vector.tensor_scalar_mul(
                    ob_all[:, it * C_OUT : (it + 1) * C_OUT],
                    po_all[:, it * 256 : it * 256 + C_OUT],
                    rc[:, it : it + 1],
                )
            else:
                nc.scalar.mul(
                    ob_all[:, it * C_OUT : (it + 1) * C_OUT],
                    po_all[:, it * 256 : it * 256 + C_OUT],
                    rc[:, it : it + 1],
                )
        outr = out.rearrange("(t p) c -> p t c", p=P)
        obr = ob_all[:].rearrange("p (t c) -> p t c", c=C_OUT)
        for tp in range(4):
            nc.sync.dma_start(
                outr[:, 2 * tp : 2 * (tp + 1), :],
                obr[:, 2 * tp : 2 * (tp + 1), :],
            )


def _prep_inputs(node_feats, adj_matrix, W, b, v0, v1):
    X = np.ascontiguousarray(node_feats, dtype=np.float32)
    W = np.asarray(W, dtype=np.float32)
    b = np.asarray(b, dtype=np.float32)
    v0 = np.asarray(v0, dtype=np.float32)
    v1 = np.asarray(v1, dtype=np.float32)

    w0l = (LAM * (W.astype(np.float64) @ v0.astype(np.float64))).astype(np.float32)
    w1l = (LAM * (W.astype(np.float64) @ v1.astype(np.float64))).astype(np.float32)
    c0l = np.float32(LAM * float(b.astype(np.float64) @ v0.astype(np.float64)))
    c1l = np.float32(LAM * float(b.astype(np.float64) @ v1.astype(np.float64)))

    XT1 = np.empty((257, N), np.float32)
    XT1[:256] = X.T
    XT1[256] = 1.0

    WE = np.zeros((257, WCOLS), np.float32)
    WE[:256, :C_OUT] = W
    WE[256, :C_OUT] = b
    WE[256, C_OUT] = 1.0          # makes h_ext column 128 identically 1
    WE[:256, C_OUT + 1] = w0l
    WE[256, C_OUT + 1] = c0l
    WE[:256, C_OUT + 2] = w1l
    WE[256, C_OUT + 2] = c1l

    XT1h = XT1.astype(np.float16)
    WEh = WE.astype(np.float16)
    A16 = np.asarray(adj_matrix, dtype=np.float16)

    in_maps = []
    for c in range(NCORES):
        in_maps.append(
            {
                "xt1": XT1h,
                "xt1l": np.ascontiguousarray(XT1h[:, c * ROWS : (c + 1) * ROWS]),
                "wext": WEh,
                "adjt": np.ascontiguousarray(
                    A16[c * ROWS : (c + 1) * ROWS, :].T
                ),
            }
        )
    return in_maps


def _run(in_maps, trace=False, b_zero=True):
    key = f"nc_b{int(b_zero)}"
    if key not in _CACHE:
        _CACHE[key] = _build_nc(b_zero=b_zero)
    nc = _CACHE[key]
    res = run_bass_kernel_spmd(
        nc, in_maps, core_ids=list(range(NCORES)), trace=trace
    )
    full = np.concatenate(
        [res.results[c]["out"] for c in range(NCORES)], axis=0
    ).astype(np.float32)
    return full, res


def kernel(node_feats, adj_matrix, W, b, v0, v1):
    in_maps = _prep_inputs(node_feats, adj_matrix, W, b, v0, v1)
    trace = bool(int(os.environ.get("GAT_TRACE", "0")))
    b_zero = not bool(np.any(np.asarray(b)))
    full, _ = _run(in_maps, trace=trace, b_zero=b_zero)
    return full
